# revision 28
# baseline (speedup 1.0000x reference)
"""BEiT attention block kernel for 8 Trainium2 NeuronCores.

Strategy: data-parallel over batch B=256 -> 32 items per core. Weights and the
(gathered, exponentiated, transposed) relative-position bias table are
replicated to every core.

v2 changes vs v1 (719us):
  - q/k projection runs in fp8-e4m3 with DoubleRow perf mode (K=256 per
    matmul, 2x PE throughput). Weights are pre-scaled x64 on the host so
    their magnitudes sit in e4m3's normal range; the extra 64*64 factor is
    divided out inside the exp (softmax is shift/scale-friendly).
  - x arrives pre-transposed from the host (bf16 copy for the v projection,
    fp8 interleaved copy for q/k) - no on-device DMA transposes.
  - output projection is token-moving (weights stationary): 7092 vs 9216
    PE cycles/item, PSUM evicted via ACT with the proj bias folded in,
    written to DRAM transposed as bf16; the host undoes the transpose.
  - v_bias is folded into the proj bias on the host (attn rows sum to 1),
    so the v eviction is a plain copy.
  - softmax reciprocal uses the fast Newton-Raphson DVE op.
  - PV PSUM tiles are paired (two heads per tile) to halve eviction count.

Per-core pipeline (item pair at a time):
  qT/kT = DoubleRow fp8 matmuls -> [1536(o), 2, 208(tok)]
          (q rows get scale*psum + scale*q_bias via ACT)
  v     = xT-stationary bf16 matmuls -> natural layout [tok, 768], augmented
          with a ones column per head -> v_aug [tok, head, 65]
  S^T   = kT_h.T @ qT_h per (head, key-chunk)  [nk, nq] in PSUM
  Pexp  = exp(S^T / 4096) * exp(biasT)  (bias folded in as a bf16 multiply)
  O^T|sums = v_aug.T @ Pexp -> [65, nq]; row 64 = softmax denominators
  normalize rows 0..63 by broadcast 1/sums, reorder to OT [768(d), 2, 197]
  out   = wp-stationary matmuls against OT -> [768(o), 2, 197] + proj_b
          -> DRAM transposed bf16
"""

import os
import sys
from contextlib import ExitStack

for _p in ("/opt/trn_rl_repo", "/opt/pypackages"):
    if os.path.isdir(_p) and _p not in sys.path:
        sys.path.append(_p)

import numpy as np
import ml_dtypes

import concourse.bacc as bacc
import concourse.bass as bass
import concourse.mybir as mybir
import concourse.tile as tile

BF16 = ml_dtypes.bfloat16
F8 = ml_dtypes.float8_e4m3

N_CORES = 8
B = 256
BC = B // N_CORES          # items per core
N = 197                    # tokens
D = 768
H = 12
DH = 64
DC = D // 128              # 6 d-chunks of 128
NQK = 2 * D                # q+k output rows
SCALE = DH ** -0.5
CH0, CH1 = 128, N - 128    # token chunks (128, 69)
CHUNKS = ((0, CH0), (CH0, CH1))
VS = 384                   # v free-dim slice (2 slices of 384 = 768)
NP = 208                   # tokens padded (16-granule; matches v1 host prep)
NP2 = 2 * NP               # pair token axis
WS = 64.0                  # host fp8 weight pre-scale (power of two)

QFP8 = True                # q projection in fp8 too (False -> only k)
GPS_MULT = True            # odd heads' exp-bias multiply on GPSIMD
GPS_NORM = True            # normalize on GPSIMD
SUMS_SBUF_DMA = True       # softmax sums respread via SBUF->SBUF DMA


def _build_body(ctx, tc, t, n_items, sim_safe=False):
    nc = tc.nc
    f32 = mybir.dt.float32
    bf16 = mybir.dt.bfloat16
    fp8 = mybir.dt.float8e4
    Ident = mybir.ActivationFunctionType.Identity
    Exp = mybir.ActivationFunctionType.Exp
    DR = mybir.MatmulPerfMode.DoubleRow
    exp_scale = 1.0 / (WS * WS) if QFP8 else 1.0 / WS

    const = ctx.enter_context(tc.tile_pool(name="const", bufs=1))
    wqk8 = const.tile([128, 3, 2, NQK if QFP8 else D], fp8)
    wq16 = None if QFP8 else const.tile([128, DC, D], bf16)
    wv = const.tile([128, DC, D], bf16)
    wp = const.tile([128, DC, D], bf16)
    qb = const.tile([128, DC], f32)
    pb = const.tile([128, DC], f32)
    ebc = const.tile([128, H, 2 * N], bf16)

    xt8p = ctx.enter_context(tc.tile_pool(name="xt8", bufs=3))
    xtvp = ctx.enter_context(tc.tile_pool(name="xtv", bufs=3))
    qkp = ctx.enter_context(tc.tile_pool(name="qk", bufs=3))
    vap = ctx.enter_context(tc.tile_pool(name="va", bufs=4))
    pep = ctx.enter_context(tc.tile_pool(name="pex", bufs=14))
    otnp = ctx.enter_context(tc.tile_pool(name="otn", bufs=2))
    rcpp = ctx.enter_context(tc.tile_pool(name="rcp", bufs=2))
    otp = ctx.enter_context(tc.tile_pool(name="ot", bufs=3))
    outp = ctx.enter_context(tc.tile_pool(name="outp", bufs=4))
    dramp = ctx.enter_context(tc.tile_pool(name="dram", bufs=2, space="DRAM"))

    ps_qk = ctx.enter_context(tc.tile_pool(name="ps_qk", bufs=2, space="PSUM"))
    ps_att = ctx.enter_context(tc.tile_pool(name="ps_att", bufs=5, space="PSUM"))
    ps_vp = ctx.enter_context(tc.tile_pool(name="ps_vp", bufs=1, space="PSUM"))

    assert n_items % 2 == 0
    n_pairs = n_items // 2

    xt8s = {}
    xtvs = {}
    qkts = {}
    vatss = {}
    otps_ = {}

    def emit_loads(g):
        """Load pair g's pre-transposed x from DRAM (fp8 pair tile + two
        bf16 item halves)."""
        xt8_g = xt8p.tile([128, 3, 2, NP2], fp8, tag="xt8", name=f"xt8{g}")
        xt8s[g] = xt8_g
        nc.sync.dma_start(
            xt8_g[:], t["xb8t"][g].rearrange("c p j u -> p c j u")
        )
        xtv_g = xtvp.tile([128, DC, 2, NP], bf16, tag="xtv", name=f"xtv{g}")
        xtvs[g] = xtv_g
        for ii in range(2):
            item = 2 * g + ii
            nc.sync.dma_start(
                xtv_g[:, :, ii, :],
                t["xb16t"][item].rearrange("c p u -> p c u"),
            )

    def emit_qk_group(g, oc):
        """One o-chunk of the qT/kT projection for pair g."""
        if oc == 0:
            qkts[g] = qkp.tile(
                [128, 2 * DC, 2, NP], bf16, tag="qkt", name=f"qkt{g}"
            )
        qkt = qkts[g]
        pq = ps_qk.tile([128, 2, NP], f32, tag="qkps")
        use_fp8 = QFP8 or oc >= DC
        if use_fp8:
            o0 = 128 * oc if QFP8 else 128 * (oc - DC)
            xt8_g = xt8s[g]
            for c in range(3):
                nc.tensor.matmul(
                    pq[:],
                    wqk8[:, c, :, o0:o0 + 128],
                    xt8_g[:, c],
                    start=(c == 0),
                    stop=(c == 2),
                    perf_mode=DR,
                )
        else:
            xtv_g = xtvs[g]
            for dc in range(DC):
                nc.tensor.matmul(
                    pq[:, :, 0:N],
                    wq16[:, dc, 128 * oc:128 * (oc + 1)],
                    xtv_g[:, dc, :, 0:N],
                    start=(dc == 0),
                    stop=(dc == DC - 1),
                )
        if oc < DC:  # q rows: scale * psum + c_q * q_bias via ACT
            if use_fp8:
                nc.scalar.activation(
                    qkt[:, oc], pq[:], Ident, bias=qb[:, oc:oc + 1], scale=SCALE
                )
            else:
                nc.scalar.activation(
                    qkt[:, oc, :, 0:N], pq[:, :, 0:N], Ident,
                    bias=qb[:, oc:oc + 1], scale=SCALE,
                )
        else:  # k rows: plain copy/cast
            nc.vector.tensor_copy(qkt[:, oc], pq[:])

    def emit_v_group(g, ii, ci, s):
        """One (item, chunk, slice) group of the v projection (6 MMs + evict)."""
        p0, pr = CHUNKS[ci]
        if ci == 0 and s == 0:
            vatss.setdefault(g, {})[ii] = []
        if s == 0:
            vat = vap.tile(
                [128, H, DH + 1], bf16, tag=f"va{ci}", name=f"va{ci}_{g}_{ii}"
            )
            nc.vector.memset(vat[0:pr, :, DH:DH + 1], 1.0)
            vatss[g][ii].append(vat)
        vat = vatss[g][ii][ci]
        xtv_g = xtvs[g]
        pv = ps_vp.tile([128, 2, NP], f32, tag="vp")
        pvf = pv.rearrange("p a b -> p (a b)")
        for dc in range(DC):
            nc.tensor.matmul(
                pvf[0:pr, 0:VS],
                xtv_g[:, dc, ii, p0:p0 + pr],
                wv[:, dc, VS * s:VS * (s + 1)],
                start=(dc == 0),
                stop=(dc == DC - 1),
            )
        nc.vector.tensor_copy(
            vat[0:pr, 6 * s:6 * (s + 1), 0:DH],
            pvf[0:pr, 0:VS].rearrange("p (h d) -> p h d", d=DH),
        )

    def emit_proj_group(g, oc, ii=None):
        """One o-chunk of the output projection for pair g (token-moving).

        ii=None: both items at once (moving 394 wide). ii given: single item
        (epilogue drains per item as soon as its OT is ready)."""
        ot_t = otps_[g]
        pp = ps_vp.tile([128, 2, NP], f32, tag="vp")
        mv = ot_t[:, :, :, :] if ii is None else ot_t[:, :, ii, :]
        wide = 2 * N if ii is None else N
        ppv = pp.rearrange("p a b -> p (a b)")
        for dc in range(DC):
            nc.tensor.matmul(
                ppv[:, 0:wide],
                wp[:, dc, 128 * oc:128 * (oc + 1)],
                mv[:, dc],
                start=(dc == 0),
                stop=(dc == DC - 1),
            )
        ob = outp.tile([128, 2 * N], bf16, tag="ob")
        nc.vector.tensor_scalar_add(ob[:, 0:wide], ppv[:, 0:wide], pb[:, oc:oc + 1])
        if ii is None:
            nc.sync.dma_start(t["y"][g, oc], ob[:].rearrange("p (i u) -> p i u", i=2))
        else:
            nc.sync.dma_start(t["y"][g, oc, :, ii, :], ob[:, 0:N])

    # ---- attention ----
    LAG = 3
    pend = {}
    otus = {}

    def emit_s(g, ii, hp):
        """S^T, exp, bias-multiply for head pair (2*hp, 2*hp+1).

        The even head streams through PE rows 0..63, the odd head through
        rows 64..127 (disjoint row groups -> the PE runs them concurrently
        and overlaps their weight loads)."""
        qkt = qkts[g]
        qc = hp
        kc = DC + hp
        tiles = {}
        for h in (2 * hp, 2 * hp + 1):
            tiles[h] = ps_att.tile([128, 2 * N], f32, tag="att", name=f"s{h}")
            if sim_safe:
                # chunk 1 only has 69 valid key rows. On HW the junk tail of
                # exp/mult is never read (the bias table zeroes the product
                # and PV only contracts rows 0:69), but the simulator's
                # uninitialized-memory checker needs it written.
                nc.vector.memset(tiles[h][64:128, N:2 * N], 0.0)
        # both chunks of both heads; even/odd heads use disjoint PE row groups
        for ci, (p0, pr) in enumerate(CHUNKS):
            for h in (2 * hp, 2 * hp + 1):
                hb = 64 * (h % 2)
                nc.tensor.matmul(
                    tiles[h][0:pr, N * ci:N * ci + N],
                    qkt[hb:hb + 64, kc, ii, p0:p0 + pr],
                    qkt[hb:hb + 64, qc, ii, 0:N],
                    start=True,
                    stop=True,
                )
        for h in (2 * hp, 2 * hp + 1):
            pex = pep.tile([128, 2 * N], bf16, tag="pex")
            nc.scalar.activation(pex[:], tiles[h][:], Exp, scale=exp_scale)
            pex2 = pep.tile([128, 2 * N], bf16, tag="pex2")
            meng = nc.gpsimd if (GPS_MULT and h % 2) else nc.vector
            meng.tensor_mul(pex2[:], pex[:], ebc[:, h, :])
            pend[(g, ii, h)] = pex2

    def emit_pv(g, ii, hp):
        if hp == 0:
            otus[(g, ii)] = otnp.tile(
                [DH + 1, H, N], f32, tag="otu", name=f"otu{g}_{ii}"
            )
        otu = otus[(g, ii)]
        po = ps_att.tile([DH + 1, 2, N], f32, tag="att")
        for hi, h in enumerate((2 * hp, 2 * hp + 1)):
            pex2 = pend.pop((g, ii, h))
            for ci, (p0, pr) in enumerate(CHUNKS):
                nc.tensor.matmul(
                    po[0:DH + 1, hi, :],
                    vatss[g][ii][ci][0:pr, h, :],
                    pex2[0:pr, N * ci:N * ci + N],
                    start=(ci == 0),
                    stop=(ci == 1),
                )
        # rows 0..63 = unnormalized O^T, row 64 = softmax denominators
        if hp % 2:
            nc.vector.tensor_copy(otu[0:DH + 1, 2 * hp:2 * hp + 2, :], po[0:DH + 1])
        else:
            nc.scalar.copy(otu[0:DH + 1, 2 * hp:2 * hp + 2, :], po[0:DH + 1])

    carry = []
    carry_late = []

    def emit_chain(g, ii):
        """Softmax denominators -> broadcast reciprocals -> reorder ->
        normalize into the (reordered, bf16) proj moving operand.

        The unnormalized O^T is reordered to [d, tok] layout as f32 first;
        the reciprocal broadcast lands in the same layout (heads map to
        partition halves), so the normalize is a single full-width multiply.
        Only the DMA hops are emitted inline; compute pieces are deferred
        into the dense-group queue."""
        otu = otus.pop((g, ii))
        sums12 = rcpp.tile([H, N], f32, tag="s12")
        if SUMS_SBUF_DMA:
            nc.sync.dma_start(sums12[:], otu[DH:DH + 1, :, :])
        else:
            dtmp = dramp.tile([1, H, N], f32, tag="drcp")
            nc.sync.dma_start(dtmp[:], otu[DH:DH + 1, :, :])
            nc.sync.dma_start(sums12[:], dtmp[0])
        rcp12 = rcpp.tile([H, N], f32, tag="r12")
        rcp_rep = rcpp.tile([128, DC, N], f32, tag="rcpr", name=f"rr{g}_{ii}")
        otr = otnp.tile([128, DC, N], f32, tag="otr", name=f"otr{g}_{ii}")
        if ii == 0:
            otps_[g] = otp.tile([128, DC, 2, N], bf16, tag="ot", name=f"ot{g}")
        ot_t = otps_[g]

        def part_recip():
            nc.vector.reciprocal_approx_fast(rcp12[:], sums12[:])
            dtmp2 = dramp.tile([H, N], f32, tag="drcp2")
            nc.sync.dma_start(dtmp2[:], rcp12[:])
            dsrc = dtmp2[:]
            # partition p of the broadcast reads head 2c + (p >= 64):
            # one DMA per partition half, dims (rep64, c, q), steps (0, 2N, 1)
            for half in range(2):
                bcast = bass.AP(
                    tensor=dsrc.tensor,
                    offset=dsrc.offset + half * N,
                    ap=[[0, 64], [2 * N, DC], [1, N]],
                )
                nc.sync.dma_start(rcp_rep[64 * half:64 * half + 64, :, :], bcast)

        # reorder depends only on otu (complete once the last PV evict ran),
        # same as the sums DMA - emit inline.
        r = otu.rearrange("p (c two) n -> p two c n", two=2)
        nc.sync.dma_start(otr[0:64, :, :], r[0:DH, 0])
        nc.sync.dma_start(otr[64:128, :, :], r[0:DH, 1])

        def part_norm():
            neng = nc.gpsimd if GPS_NORM else nc.vector
            neng.tensor_mul(ot_t[:, :, ii, :], otr[:], rcp_rep[:])

        # the reciprocal + broadcast DMAs pop early next pair; the normalize
        # (which needs the broadcast roundtrip done) pops mid-pair so the
        # compute queues never stall waiting on it.
        carry.append(part_recip)
        carry_late.append(part_norm)

    def dense_groups_for(g):
        """Dense PE work interleaved into pair (g-1)'s attention: qkT(g),
        v(g), proj(g-2) (its chain resolved early this pair)."""
        qk_l, proj_l, v_l = [], [], []
        if g < n_pairs:
            # q/k interleaved so next pair's S units (which need q-chunk hp
            # AND k-chunk hp) see their stationaries evicted in unit order
            for hp in range(DC):
                qk_l.append(lambda oc=hp: emit_qk_group(g, oc))
                qk_l.append(lambda oc=DC + hp: emit_qk_group(g, oc))
        if g - 2 >= 0:
            for oc in range(DC):
                proj_l.append(lambda oc=oc: emit_proj_group(g - 2, oc))
        if g < n_pairs:
            for ii in range(2):
                for ci in range(2):
                    for s in range(2):
                        v_l.append(
                            lambda ii=ii, ci=ci, s=s: emit_v_group(g, ii, ci, s)
                        )
        return qk_l, proj_l, v_l

    # ---- prologue ----
    # pair-0 loads + the weights needed first go to the DMA queues first; the
    # rest of the constants load while the first qkT matmuls run
    # HAM warm-up: the PE clock-gate needs ~3.4us of sustained activity to
    # lift K=4/8 -> 8/8. Burn junk matmuls into a dead PSUM tile while the
    # first loads are in flight so the first real pair runs at full clock.
    scr = const.tile([128, 512], bf16)
    nc.vector.memset(scr[:], 1.0)
    pwarm = ps_qk.tile([128, 2, NP], f32, tag="qkps")
    pwv = pwarm.rearrange("p a b -> p (a b)")
    for _ in range(30):
        nc.tensor.matmul(
            pwv[0:64, 0:NP2], scr[:, 0:64], scr[:, 0:NP2], start=True, stop=True
        )

    # first qk matmuls need only xt8(0) + their own wqk8 o-slice: stage the
    # weight DMAs so oc {0,6,1,7} unblock early, and emit qk0 q/k-interleaved
    # so the S chase can start as soon as possible
    nc.sync.dma_start(qb[:], t["qb"])
    emit_loads(0)
    if QFP8:
        nc.sync.dma_start(wqk8[:, :, :, 0:256], t["wqk8"][:, :, :, 0:256])
        nc.sync.dma_start(wqk8[:, :, :, D:D + 256], t["wqk8"][:, :, :, D:D + 256])
    else:
        nc.sync.dma_start(wq16[:], t["wq16"])
        nc.sync.dma_start(wqk8[:, :, :, 0:256], t["wqk8"][:, :, :, 0:256])
    _, _, v0 = dense_groups_for(0)
    for oc in (0, DC, 1, DC + 1):
        emit_qk_group(0, oc)
    if QFP8:
        nc.sync.dma_start(wqk8[:, :, :, 256:D], t["wqk8"][:, :, :, 256:D])
        nc.sync.dma_start(
            wqk8[:, :, :, D + 256:NQK], t["wqk8"][:, :, :, D + 256:NQK]
        )
    else:
        nc.sync.dma_start(wqk8[:, :, :, 256:D], t["wqk8"][:, :, :, 256:D])
    for oc in (2, DC + 2, 3, DC + 3, 4, DC + 4, 5, DC + 5):
        emit_qk_group(0, oc)
    nc.sync.dma_start(wv[:], t["wv"])
    nc.sync.dma_start(ebc[:], t["ebc"])
    nc.sync.dma_start(wp[:], t["wp"])
    nc.sync.dma_start(pb[:], t["pb"])
    emit_loads(1)
    for fn in v0:
        fn()

    # ---- steady state: per-pair attention with dense work interleaved ----
    deferred_rr = []
    for g in range(n_pairs):
        units = [(ii, hp) for ii in range(2) for hp in range(H // 2)]
        qk_l, proj_l, v_l = dense_groups_for(g + 1)
        # round-robin LDWEIGHTS-bound qk groups with stream-bound v groups to
        # keep PE duty even (HAM watches a 3.4us activity window); proj last
        # (it must be emitted after the normalizes that produce its input).
        rr = []
        for k in range(max(len(qk_l), len(v_l))):
            if k < len(qk_l):
                rr.append(qk_l[k])
            if k < len(v_l):
                rr.append(v_l[k])
        if g == n_pairs - 2 and n_pairs >= 2:
            # hold back the second half of the last pair's dense work so the
            # final iteration (which has no further qk/v) keeps the PE fed
            deferred_rr = rr[12:]
            rr = rr[:12]
            groups = carry[:] + rr + carry_late[:] + proj_l
        elif g == n_pairs - 1:
            groups = carry[:] + deferred_rr + carry_late[:] + proj_l
            deferred_rr = []
        else:
            groups = carry[:] + rr[:12] + carry_late[:] + rr[12:] + proj_l
        if g + 2 <= n_pairs - 1:
            # prefetch two pairs ahead, after the carried chain DMAs so the
            # small reciprocal hops aren't queued behind 600KB of loads
            groups.insert(len(carry), lambda g=g: emit_loads(g + 2))
        carry.clear()
        carry_late.clear()
        gi = 0
        for idx in range(len(units) + LAG):
            if idx < len(units):
                ii, hp = units[idx]
                emit_s(g, ii, hp)
            if idx >= LAG:
                ii, hp = units[idx - LAG]
                emit_pv(g, ii, hp)
                if hp == H // 2 - 1:
                    emit_chain(g, ii)
            if g == n_pairs - 1 and idx == LAG + 5:
                # final pair: drain item 0's chain + half its projections
                # inside the loop so the PE isn't starved (and HAM-throttled)
                # at the end; the other half bridges item 1's chain latency
                groups.append(carry.pop(0))
                groups.append(carry_late.pop(0))
                for oc in range(DC // 2):
                    groups.append(lambda oc=oc: emit_proj_group(g, oc, ii=0))
            for _ in range(2):
                if gi < len(groups):
                    groups[gi]()
                    gi += 1
        while gi < len(groups):
            groups[gi]()
            gi += 1

    # ---- epilogue: item 1's chain latency is bridged by item 0's
    # remaining projections, then item 1's projections drain ----
    for fn in carry:
        fn()
    carry.clear()
    for oc in range(DC // 2, DC):
        emit_proj_group(n_pairs - 1, oc, ii=0)
    for fn in carry_late:
        fn()
    for oc in range(DC):
        emit_proj_group(n_pairs - 1, oc, ii=1)
    carry_late.clear()


def build_program(n_items=BC, enable_asserts=False):
    nc = bacc.Bacc(
        "TRN2",
        target_bir_lowering=False,
        debug=False,
        enable_asserts=enable_asserts,
        num_devices=1,
    )
    f32 = mybir.dt.float32
    bf16 = mybir.dt.bfloat16
    fp8 = mybir.dt.float8e4
    n_pairs = n_items // 2
    t = {
        "xb16t": nc.dram_tensor(
            "xb16t", [n_items, DC, 128, NP], bf16, kind="ExternalInput"
        ).ap(),
        "xb8t": nc.dram_tensor(
            "xb8t", [n_pairs, 3, 128, 2, NP2], fp8, kind="ExternalInput"
        ).ap(),
        "wqk8": nc.dram_tensor(
            "wqk8", [128, 3, 2, NQK if QFP8 else D], fp8, kind="ExternalInput"
        ).ap(),
        "wv": nc.dram_tensor("wv", [128, DC, D], bf16, kind="ExternalInput").ap(),
        "wp": nc.dram_tensor("wp", [128, DC, D], bf16, kind="ExternalInput").ap(),
        "qb": nc.dram_tensor("qb", [128, DC], f32, kind="ExternalInput").ap(),
        "pb": nc.dram_tensor("pb", [128, DC], f32, kind="ExternalInput").ap(),
        "ebc": nc.dram_tensor(
            "ebc", [128, H, 2 * N], bf16, kind="ExternalInput"
        ).ap(),
        "y": nc.dram_tensor(
            "y", [n_pairs, DC, 128, 2, N], bf16, kind="ExternalOutput"
        ).ap(),
    }
    if not QFP8:
        t["wq16"] = nc.dram_tensor(
            "wq16", [128, DC, D], bf16, kind="ExternalInput"
        ).ap()
    with tile.TileContext(nc) as tc:
        with ExitStack() as ctx:
            _build_body(ctx, tc, t, n_items, sim_safe=enable_asserts)
    nc.compile()
    return nc


def host_constants(qkv_w, q_bias, v_bias, rel_pos_table, proj_w, proj_b, rel_index):
    qkv_w = np.asarray(qkv_w, np.float32)
    proj_w = np.asarray(proj_w, np.float32)
    q_bias = np.asarray(q_bias, np.float32)
    v_bias = np.asarray(v_bias, np.float32)
    proj_b = np.asarray(proj_b, np.float32)
    rel_pos_table = np.asarray(rel_pos_table, np.float32)
    rel_index = np.asarray(rel_index)

    wt = qkv_w.T  # [768, 2304]
    nqk = NQK if QFP8 else D
    w8src = wt[:, :NQK] if QFP8 else wt[:, D:NQK]
    wqk8 = np.clip(WS * w8src, -240, 240).reshape(3, 128, 2, nqk)
    wqk8 = np.ascontiguousarray(wqk8.transpose(1, 0, 2, 3)).astype(F8)
    wv = wt[:, NQK:].reshape(DC, 128, D).transpose(1, 0, 2).astype(BF16)
    wp = proj_w.T.reshape(DC, 128, D).transpose(1, 0, 2).astype(BF16)
    cq = (WS * SCALE) if QFP8 else SCALE
    qb = np.ascontiguousarray((cq * q_bias).reshape(DC, 128).T)
    pbp = proj_b + proj_w @ v_bias    # fold v_bias through the projection
    pb = np.ascontiguousarray(pbp.reshape(DC, 128).T)
    # bias[q, k, h] -> exp -> [h, k, q] (transposed for the S^T layout);
    # chunk 1 rows 69..127 stay zero so junk exp values are masked out
    ebT = np.exp(rel_pos_table[rel_index].astype(np.float64)).transpose(2, 1, 0)
    ebc = np.zeros((128, H, 2 * N), np.float64)
    ebc[:CH0, :, :N] = ebT[:, :CH0, :].transpose(1, 0, 2)
    ebc[:CH1, :, N:] = ebT[:, CH0:, :].transpose(1, 0, 2)
    ebc = ebc.astype(BF16)
    out = {
        "wqk8": wqk8, "wv": wv, "wp": wp, "qb": qb, "pb": pb, "ebc": ebc,
    }
    if not QFP8:
        out["wq16"] = (
            wt[:, :D].reshape(DC, 128, D).transpose(1, 0, 2).astype(BF16)
        )
    return out


def host_x(x):
    """Pre-transposed x layouts: bf16 [B, DC, 128, NP] and fp8 pair tiles
    [B//2, 3, 128, 2, 2*NP]."""
    b = x.shape[0]
    xb16 = np.zeros((b, NP, D), BF16)
    xb16[:, :N, :] = np.asarray(x, np.float32).astype(BF16)
    xt = xb16.transpose(0, 2, 1)                        # [b, 768, NP]
    xb16t = np.ascontiguousarray(xt.reshape(b, DC, 128, NP))
    x8 = xt.astype(F8).reshape(b // 2, 2, 3, 128, 2, NP)
    xb8t = np.ascontiguousarray(
        x8.transpose(0, 2, 3, 4, 1, 5).reshape(b // 2, 3, 128, 2, NP2)
    )
    return xb16t, xb8t


_PROG_CACHE = {}


def get_program(n_items=BC):
    if n_items not in _PROG_CACHE:
        _PROG_CACHE[n_items] = build_program(n_items)
    return _PROG_CACHE[n_items]


def assemble_out(y_cores):
    """[cores][n_pairs, DC, 128, 2, N] bf16 -> [B, N, D] f32."""
    y = np.concatenate([np.asarray(yc) for yc in y_cores], axis=0)
    out = y.transpose(0, 3, 4, 1, 2).reshape(B, N, D)
    return np.ascontiguousarray(out.astype(np.float32))


def run(inputs, trace=False):
    """Run on all 8 cores. Returns (output [256,197,768] f32, exec_time_ns|None)."""
    from concourse.bass_utils import run_bass_kernel_spmd

    x = np.asarray(inputs["x"], np.float32)
    consts = host_constants(
        inputs["qkv_w"], inputs["q_bias"], inputs["v_bias"],
        inputs["rel_pos_table"], inputs["proj_w"], inputs["proj_b"],
        inputs["rel_index"],
    )
    nc = get_program(BC)
    xb16t, xb8t = host_x(x)
    pc = BC // 2
    in_maps = [
        {
            "xb16t": xb16t[c * BC:(c + 1) * BC],
            "xb8t": xb8t[c * pc:(c + 1) * pc],
            **consts,
        }
        for c in range(N_CORES)
    ]
    res = run_bass_kernel_spmd(
        nc, in_maps, core_ids=list(range(N_CORES)), trace=trace
    )
    out = assemble_out([res.results[c]["y"] for c in range(N_CORES)])
    return out, res.exec_time_ns


def kernel(**inputs) -> np.ndarray:
    out, _ = run(inputs, trace=False)
    return out
